# revision 20
# baseline (speedup 1.0000x reference)
"""Pairwise squared-euclidean distance kernel for Trainium2 (8 NeuronCores).

z[i, j] = ||x_i||^2 + ||y_j||^2 - 2 * <x_i, y_j>

Sharding (v16): 2D 4x2 grid. x rows split across 4 row-groups (2048 each),
y columns split across 2 col-groups (4096 each). Each core computes a
[2048, 4096] tile of the output with no communication. Per-core HBM
traffic: in 6MB (x 2MB + yT 4MB) out 16.8MB (fp16) -> ~64us BW floor at
358 GB/s per core.

Host prep inside kernel(): y is transposed once on the host (pure layout,
no FLOPs moved off-device) so each core DMA-loads yT [256, 4096] fp32
directly in the [d, j] layout the PE needs - no y transposes on device.

Per-core pipeline:
  1. Loads, ALL on the sync HWDGE ring serialized in priority order
     (x0, y0, y1, x1, x2, x3, y2, y3) followed by the out stream -
     concurrent rings round-robin at packet level and starve the
     critical path. x uses a row-permuted "(p t)" layout (partition p
     holds rows 16p..16p+15) so every DMA line is 4KB; the output AP
     uses the same permutation so results land in the right rows.
  2. x: ScalarE casts bf16*(-2), PE transposes (identity matmul, one
     [128,2,512] PSUM tile per 4-tile chunk), ScalarE evacs -> fp8e4
     xT8 in one wide op. x row-norms: ScalarE Square+accum singles for
     tiles 0-3 (head), DVE square/tensor_reduce(axis=X) batches for
     tiles 4-15 (quarter-0 inserts).
  3. y: fp32 chunks [128, 2, 1024]; one wide ScalarE fp32->fp8 quant
     per chunk; DVE yTsq = yT0^2 + yT1^2 bf16.
  4. Main loop: 32 groups (2 y-halves x 16 m-tiles). Per group TWO
     2-bank PSUM tiles [128, 1024] fp32 (one per evac engine - a
     shared tile serializes the evacs through Tile deps). Per group
     4 fp8 DoubleRow matmuls + 4 bf16 ones-matmuls accumulating
     ysq[j] (the rank-1 term rides the PE; evac-side adds would swamp
     DVE). ~1.75us/group = the N=512 streaming limit. Prep for x
     chunks 1-3 / y chunks 2-3 is spread one op per group over q0,
     meeting per-tile deadlines; x transposes for chunks 1-3 are
     emitted as inserts so the PE's in-order queue never blocks the
     next group's matmuls on a pending cast.
  5. Evac: ScalarE activation(psumA + xsq bias) -> fp16, DVE
     tensor_scalar_add(psumB + xsq) -> fp16, in parallel. Host
     upcasts fp16 -> fp32.

Known-good environment notes: tensor_tensor_reduce crashes the device.
fp32 XBAR wedges the device; SWDGE cast-DMA races consumers; gpsimd
compute interferes with DVE SBUF ports - all avoided. DVE fp8-out is
~3x slow - fp8 stores only via ScalarE. ScalarE accum_out is scalar
only and costs a hidden ~280ns READ_ACCUMULATOR per op.
"""

import os

import numpy as np

import concourse.bacc as bacc
import concourse.mybir as mybir
import concourse.tile as tile
from concourse.bass_utils import run_bass_kernel_spmd
from concourse.masks import make_identity

N_CORES = 8
N_FULL = 8192  # total x rows
M_FULL = 8192  # total y rows
D = 256  # feature dim

R_SHARDS = 4  # x-row shards
C_SHARDS = 2  # y-col shards
N_SHARD = N_FULL // R_SHARDS  # 2048 x rows per core
M_SHARD = M_FULL // C_SHARDS  # 4096 y cols per core

P = 128
NT = 512  # one fp32 PSUM bank
GRP = 4  # PSUM banks per group
QCOLS = GRP * NT  # 2048
Q = M_SHARD // QCOLS  # 2 y halves
M_TILES = N_SHARD // P  # 16
YCH = 1024  # y cols per load chunk
XCH_T = 4  # x m-tiles per load/cast/transpose chunk
N_XCH = M_TILES // XCH_T  # 4

FP32 = mybir.dt.float32
BF16 = mybir.dt.bfloat16
FP16 = mybir.dt.float16
FP8 = mybir.dt.float8e4
DR = mybir.MatmulPerfMode.DoubleRow
AF = mybir.ActivationFunctionType
ALU = mybir.AluOpType

_CACHE = {}
LAST_RESULTS = None


def _build():
    nc = bacc.Bacc("TRN2", target_bir_lowering=False, debug=False, num_devices=N_CORES)
    x_d = nc.dram_tensor("x", [N_SHARD, D], FP32, kind="ExternalInput").ap()
    yt_d = nc.dram_tensor("yt", [D, M_SHARD], FP32, kind="ExternalInput").ap()
    out_d = nc.dram_tensor("out", [N_SHARD, M_SHARD], FP16, kind="ExternalOutput").ap()

    # row-permuted views: partition p holds x rows 16p..16p+15; "tile" t
    # is row 16p+t. The output uses the same permutation so group (q, t)
    # writes rows {16p+t} - 4KB DMA lines everywhere.
    x_r = x_d.rearrange("(p t) d -> p t d", p=P)
    out_r = out_d.rearrange("(p t) c -> p t c", p=P)

    with tile.TileContext(nc) as tc:
        with (
            tc.tile_pool(name="const", bufs=1) as const,
            tc.tile_pool(name="sq", bufs=6) as sqp,
            tc.tile_pool(name="ystage", bufs=4) as ystage,
            tc.tile_pool(name="outp", bufs=6) as outp,
            tc.tile_pool(name="psmm", bufs=4, space="PSUM") as psmm,
        ):
            ones = const.tile([P, P], BF16)
            nc.vector.memset(ones[:], 1.0)
            identity = const.tile([P, P], BF16)
            make_identity(nc, identity)

            xsq = const.tile([P, M_TILES], FP32)
            x_nat = const.tile([P, M_TILES, D], FP32)
            xbf = const.tile([P, M_TILES, D], BF16)
            xT8 = const.tile([P, 2, N_SHARD], FP8, name="xT8")
            yT8 = const.tile([P, 2, M_SHARD], FP8, name="yT8")
            yTsq = const.tile([P, M_SHARD], BF16, name="yTsq")

            # ---- x pieces ----
            def x_load(xc, eng=None):
                sl = slice(xc * XCH_T, (xc + 1) * XCH_T)
                (eng or nc.sync).dma_start(x_nat[:, sl, :], x_r[:, sl, :])

            def x_cast(xc):
                sl = slice(xc * XCH_T, (xc + 1) * XCH_T)
                nc.scalar.activation(
                    xbf[:, sl, :], x_nat[:, sl, :], AF.Identity, scale=-2.0
                )

            def x_tr(xc):
                # both 128-d halves of 4 m-tiles -> one [P, 2, 512] PSUM
                # tile -> one wide fp8 evac
                ps = psmm.tile([P, 2, XCH_T * P], BF16, tag="mm", name=f"xtr{xc}")
                for c in range(2):
                    for t in range(XCH_T):
                        nc.tensor.transpose(
                            ps[:, c, t * P : (t + 1) * P],
                            xbf[:, xc * XCH_T + t, c * P : (c + 1) * P],
                            identity,
                        )
                cols = slice(xc * XCH_T * P, (xc + 1) * XCH_T * P)
                nc.scalar.copy(xT8[:, :, cols], ps[:])

            def x_sq(t):
                # ScalarE single-tile row norm (exact fp32)
                sq = sqp.tile([P, D], FP32, tag="sq")
                nc.scalar.activation(
                    sq[:], x_nat[:, t, :], AF.Square, accum_out=xsq[:, t : t + 1]
                )

            def x_sqb(xc):
                # DVE batched row norms from the bf16 (-2x) tiles
                sl = slice(xc * XCH_T, (xc + 1) * XCH_T)
                sq = sqp.tile([P, XCH_T, D], BF16, tag="sqv")
                nc.vector.tensor_tensor(sq[:], xbf[:, sl, :], xbf[:, sl, :], ALU.mult)
                x4 = sqp.tile([P, XCH_T], FP32, tag="x4")
                nc.vector.tensor_reduce(
                    x4[:], sq[:], axis=mybir.AxisListType.X, op=ALU.add
                )
                nc.vector.tensor_scalar_mul(xsq[:, sl], x4[:], 0.25)

            # ---- y chunk staging ----
            ystg = {}

            def y_load(ch, eng=None):
                cols = slice(ch * YCH, (ch + 1) * YCH)
                yst = ystage.tile([P, 2, YCH], FP32, tag="yst")
                (eng or nc.sync).dma_start(
                    yst[:], yt_d[:, cols].rearrange("(h p) c -> p h c", p=P)
                )
                ystg[ch] = yst

            def y_quant(ch, half=None):
                if half is None:
                    cols = slice(ch * YCH, (ch + 1) * YCH)
                    nc.scalar.copy(yT8[:, :, cols], ystg[ch][:])
                else:
                    c0 = half * (YCH // 2)
                    cols = slice(ch * YCH + c0, ch * YCH + c0 + YCH // 2)
                    nc.scalar.copy(
                        yT8[:, :, cols], ystg[ch][:, :, c0 : c0 + YCH // 2]
                    )

            _tsq_tmp = {}

            def y_tsq(ch, step):
                if step < 2:
                    yst = ystg[ch]
                    t = sqp.tile([P, YCH], BF16, tag=f"t{step}")
                    nc.vector.tensor_tensor(
                        t[:], yst[:, step, :], yst[:, step, :], ALU.mult
                    )
                    _tsq_tmp[(ch, step)] = t
                else:
                    cols = slice(ch * YCH, (ch + 1) * YCH)
                    nc.vector.tensor_tensor(
                        yTsq[:, cols],
                        _tsq_tmp.pop((ch, 0))[:],
                        _tsq_tmp.pop((ch, 1))[:],
                        ALU.add,
                    )

            # ---- head ----
            # The two most latency-critical loads ride SEPARATE HWDGE
            # rings so their cold-start latencies overlap; everything
            # else serializes behind them on the sync ring (late loads
            # on a second ring would steal bandwidth from the critical
            # path - v14 lesson).
            x_load(0, nc.sync)
            y_load(0, nc.scalar)
            x_load(1, nc.scalar)
            y_load(1, nc.sync)
            x_load(2, nc.sync)
            x_load(3, nc.sync)
            y_load(2, nc.sync)
            y_load(3, nc.sync)
            # PE HAM warm-up: ~3.4us of dummy matmuls in the idle window
            # before data arrives, so transposes + first groups run at
            # 2.4GHz instead of the cold 1.2GHz default. Output discarded.
            warm = psmm.tile([P, P], FP32, tag="mm", name="warm")
            for _ in range(32):
                nc.tensor.matmul(warm[:], ones[:], identity[:], start=True, stop=True)
            # ScalarE chain in dependency-arrival order; quants of the
            # first chunks split 512-col so the first DR isn't gated on
            # a 1.9us wide op.
            x_cast(0)
            y_quant(0, 0)
            y_quant(0, 1)
            x_tr(0)
            y_quant(1, 0)
            y_quant(1, 1)
            x_sq(0)
            x_sq(1)
            # DVE head chain:
            y_tsq(0, 0)
            y_tsq(0, 1)
            y_tsq(0, 2)
            y_tsq(1, 0)
            y_tsq(1, 1)
            y_tsq(1, 2)

            # q0 per-group prep inserts. Deadlines: xT8 m4-7 by g4,
            # m8-11 by g8, m12-15 by g12; xsq[m] by g(m); yT8/yTsq
            # ch2-3 by g16. sqb/tsq on DVE, the rest ScalarE.
            # NOTE: a reader emitted BEFORE its writer in program order
            # reads garbage silently (Tile only orders against already-
            # emitted writers) - every sqb/tr must come after its cast.
            inserts = {
                1: [("sq", 2), ("sq", 3)],
                2: [("cast", 1)],
                3: [("tr", 1), ("sqb", 1)],
                4: [("cast", 2)],
                5: [("tr", 2), ("tsq", 2, 0)],
                6: [("sq", 8), ("tsq", 2, 1)],
                7: [("sq", 9), ("tsq", 2, 2)],
                8: [("sq", 10), ("cast", 3)],
                9: [("sq", 11), ("tr", 3)],
                10: [("sqb", 3)],
                11: [("quant", 2), ("tsq", 3, 0)],
                12: [("tsq", 3, 1)],
                13: [("quant", 3), ("tsq", 3, 2)],
            }

            def run_insert(ins):
                kind = ins[0]
                if kind == "cast":
                    x_cast(ins[1])
                elif kind == "sq":
                    x_sq(ins[1])
                elif kind == "sqb":
                    x_sqb(ins[1])
                elif kind == "tr":
                    x_tr(ins[1])
                elif kind == "quant":
                    y_quant(ins[1])
                elif kind == "tsq":
                    y_tsq(ins[1], ins[2])

            # ---- main loop ----
            for q in range(Q):
                for m in range(M_TILES):
                    if q == 0:
                        for ins in inserts.get(m, []):
                            run_insert(ins)
                    lhs8 = xT8[:, :, m * P : (m + 1) * P]
                    pmA = psmm.tile([P, 2 * NT], FP32, tag="mm", name=f"pa_{q}_{m}")
                    pmB = psmm.tile([P, 2 * NT], FP32, tag="mm", name=f"pb_{q}_{m}")
                    for k in range(GRP):
                        n = q * GRP + k
                        pm = pmA if k < 2 else pmB
                        nc.tensor.matmul(
                            pm[:, (k % 2) * NT : (k % 2 + 1) * NT],
                            lhs8,
                            yT8[:, :, n * NT : (n + 1) * NT],
                            perf_mode=DR,
                            start=True,
                            stop=False,
                        )
                    for k in range(GRP):
                        n = q * GRP + k
                        pm = pmA if k < 2 else pmB
                        nc.tensor.matmul(
                            pm[:, (k % 2) * NT : (k % 2 + 1) * NT],
                            ones[:],
                            yTsq[:, n * NT : (n + 1) * NT],
                            start=False,
                            stop=True,
                        )
                    ot = outp.tile([P, QCOLS], FP16, tag="ot")
                    nc.scalar.activation(
                        ot[:, : 2 * NT],
                        pmA[:],
                        AF.Identity,
                        bias=xsq[:, m : m + 1],
                        scale=1.0,
                    )
                    nc.vector.tensor_scalar_add(
                        ot[:, 2 * NT :], pmB[:], xsq[:, m : m + 1]
                    )
                    nc.sync.dma_start(
                        out_r[:, m, q * QCOLS : (q + 1) * QCOLS], ot[:]
                    )

    nc.compile()
    return nc


def _get_nc():
    if "nc" not in _CACHE:
        _CACHE["nc"] = _build()
    return _CACHE["nc"]


def kernel(x: np.ndarray, y: np.ndarray) -> np.ndarray:
    global LAST_RESULTS
    x = np.ascontiguousarray(np.asarray(x, dtype=np.float32))
    y = np.ascontiguousarray(np.asarray(y, dtype=np.float32))
    assert x.shape == (N_FULL, D) and y.shape == (M_FULL, D)

    nc = _get_nc()
    yt = y.T  # [D, M_FULL], layout prep only
    yhalves = [
        np.ascontiguousarray(yt[:, c * M_SHARD : (c + 1) * M_SHARD])
        for c in range(C_SHARDS)
    ]
    in_maps = []
    for core in range(N_CORES):
        r, c = divmod(core, C_SHARDS)
        in_maps.append({"x": x[r * N_SHARD : (r + 1) * N_SHARD], "yt": yhalves[c]})
    res = run_bass_kernel_spmd(
        nc,
        in_maps,
        core_ids=list(range(N_CORES)),
        trace=bool(os.environ.get("BASS_KERNEL_TRACE")),
    )
    LAST_RESULTS = res
    out = np.empty((N_FULL, M_FULL), dtype=np.float32)
    for core in range(N_CORES):
        r, c = divmod(core, C_SHARDS)
        out[r * N_SHARD : (r + 1) * N_SHARD, c * M_SHARD : (c + 1) * M_SHARD] = (
            res.results[core]["out"].astype(np.float32)
        )
    return out


# revision 21
# speedup vs baseline: 1.0187x; 1.0187x over previous
"""Pairwise squared-euclidean distance kernel for Trainium2 (8 NeuronCores).

z[i, j] = ||x_i||^2 + ||y_j||^2 - 2 * <x_i, y_j>

Sharding (v16): 2D 4x2 grid. x rows split across 4 row-groups (2048 each),
y columns split across 2 col-groups (4096 each). Each core computes a
[2048, 4096] tile of the output with no communication. Per-core HBM
traffic: in 6MB (x 2MB + yT 4MB) out 16.8MB (fp16) -> ~64us BW floor at
358 GB/s per core.

Host prep inside kernel(): y is transposed once on the host (pure layout,
no FLOPs moved off-device) so each core DMA-loads yT [256, 4096] fp32
directly in the [d, j] layout the PE needs - no y transposes on device.

Per-core pipeline:
  1. Loads, ALL on the sync HWDGE ring serialized in priority order
     (x0, y0, y1, x1, x2, x3, y2, y3) followed by the out stream -
     concurrent rings round-robin at packet level and starve the
     critical path. x uses a row-permuted "(p t)" layout (partition p
     holds rows 16p..16p+15) so every DMA line is 4KB; the output AP
     uses the same permutation so results land in the right rows.
  2. x: ScalarE casts bf16*(-2), PE transposes (identity matmul, one
     [128,2,512] PSUM tile per 4-tile chunk), ScalarE evacs -> fp8e4
     xT8 in one wide op. x row-norms: ScalarE Square+accum singles for
     tiles 0-3 (head), DVE square/tensor_reduce(axis=X) batches for
     tiles 4-15 (quarter-0 inserts).
  3. y: fp32 chunks [128, 2, 1024]; one wide ScalarE fp32->fp8 quant
     per chunk; DVE yTsq = yT0^2 + yT1^2 bf16.
  4. Main loop: 32 groups (2 y-halves x 16 m-tiles). Per group TWO
     2-bank PSUM tiles [128, 1024] fp32 (one per evac engine - a
     shared tile serializes the evacs through Tile deps). Per group
     4 fp8 DoubleRow matmuls + 4 bf16 ones-matmuls accumulating
     ysq[j] (the rank-1 term rides the PE; evac-side adds would swamp
     DVE). ~1.75us/group = the N=512 streaming limit. Prep for x
     chunks 1-3 / y chunks 2-3 is spread one op per group over q0,
     meeting per-tile deadlines; x transposes for chunks 1-3 are
     emitted as inserts so the PE's in-order queue never blocks the
     next group's matmuls on a pending cast.
  5. Evac: ScalarE activation(psumA + xsq bias) -> fp16, DVE
     tensor_scalar_add(psumB + xsq) -> fp16, in parallel. Host
     upcasts fp16 -> fp32.

Known-good environment notes: tensor_tensor_reduce crashes the device.
fp32 XBAR wedges the device; SWDGE cast-DMA races consumers; gpsimd
compute interferes with DVE SBUF ports - all avoided. DVE fp8-out is
~3x slow - fp8 stores only via ScalarE. ScalarE accum_out is scalar
only and costs a hidden ~280ns READ_ACCUMULATOR per op.
"""

import os

import numpy as np

import concourse.bacc as bacc
import concourse.mybir as mybir
import concourse.tile as tile
from concourse.bass_utils import run_bass_kernel_spmd
from concourse.masks import make_identity

N_CORES = 8
N_FULL = 8192  # total x rows
M_FULL = 8192  # total y rows
D = 256  # feature dim

R_SHARDS = 4  # x-row shards
C_SHARDS = 2  # y-col shards
N_SHARD = N_FULL // R_SHARDS  # 2048 x rows per core
M_SHARD = M_FULL // C_SHARDS  # 4096 y cols per core

P = 128
NT = 512  # one fp32 PSUM bank
GRP = 4  # PSUM banks per group
QCOLS = GRP * NT  # 2048
Q = M_SHARD // QCOLS  # 2 y halves
M_TILES = N_SHARD // P  # 16
YCH = 1024  # y cols per load chunk
XCH_T = 4  # x m-tiles per load/cast/transpose chunk
N_XCH = M_TILES // XCH_T  # 4

FP32 = mybir.dt.float32
BF16 = mybir.dt.bfloat16
FP16 = mybir.dt.float16
FP8 = mybir.dt.float8e4
DR = mybir.MatmulPerfMode.DoubleRow
AF = mybir.ActivationFunctionType
ALU = mybir.AluOpType

_CACHE = {}
LAST_RESULTS = None


def _build():
    nc = bacc.Bacc("TRN2", target_bir_lowering=False, debug=False, num_devices=N_CORES)
    x_d = nc.dram_tensor("x", [N_SHARD, D], FP32, kind="ExternalInput").ap()
    yt_d = nc.dram_tensor("yt", [D, M_SHARD], FP32, kind="ExternalInput").ap()
    out_d = nc.dram_tensor("out", [N_SHARD, M_SHARD], FP16, kind="ExternalOutput").ap()

    # row-permuted views: partition p holds x rows 16p..16p+15; "tile" t
    # is row 16p+t. The output uses the same permutation so group (q, t)
    # writes rows {16p+t} - 4KB DMA lines everywhere.
    x_r = x_d.rearrange("(p t) d -> p t d", p=P)
    out_r = out_d.rearrange("(p t) c -> p t c", p=P)

    with tile.TileContext(nc) as tc:
        with (
            tc.tile_pool(name="const", bufs=1) as const,
            tc.tile_pool(name="sq", bufs=6) as sqp,
            tc.tile_pool(name="ystage", bufs=4) as ystage,
            tc.tile_pool(name="outp", bufs=6) as outp,
            tc.tile_pool(name="psmm", bufs=4, space="PSUM") as psmm,
        ):
            ones = const.tile([P, P], BF16)
            nc.vector.memset(ones[:], 1.0)
            identity = const.tile([P, P], BF16)
            make_identity(nc, identity)

            xsq = const.tile([P, M_TILES], FP32)
            x_nat = const.tile([P, M_TILES, D], FP32)
            xbf = const.tile([P, M_TILES, D], BF16)
            xT8 = const.tile([P, 2, N_SHARD], FP8, name="xT8")
            yT8 = const.tile([P, 2, M_SHARD], FP8, name="yT8")
            yTsq = const.tile([P, M_SHARD], BF16, name="yTsq")

            # ---- x pieces ----
            def x_load(xc, eng=None):
                sl = slice(xc * XCH_T, (xc + 1) * XCH_T)
                (eng or nc.sync).dma_start(x_nat[:, sl, :], x_r[:, sl, :])

            def x_cast(xc):
                sl = slice(xc * XCH_T, (xc + 1) * XCH_T)
                nc.scalar.activation(
                    xbf[:, sl, :], x_nat[:, sl, :], AF.Identity, scale=-2.0
                )

            def x_tr(xc):
                # both 128-d halves of 4 m-tiles -> one [P, 2, 512] PSUM
                # tile -> one wide fp8 evac
                ps = psmm.tile([P, 2, XCH_T * P], BF16, tag="mm", name=f"xtr{xc}")
                for c in range(2):
                    for t in range(XCH_T):
                        nc.tensor.transpose(
                            ps[:, c, t * P : (t + 1) * P],
                            xbf[:, xc * XCH_T + t, c * P : (c + 1) * P],
                            identity,
                        )
                cols = slice(xc * XCH_T * P, (xc + 1) * XCH_T * P)
                nc.scalar.copy(xT8[:, :, cols], ps[:])

            def x_sq(t):
                # ScalarE single-tile row norm (exact fp32)
                sq = sqp.tile([P, D], FP32, tag="sq")
                nc.scalar.activation(
                    sq[:], x_nat[:, t, :], AF.Square, accum_out=xsq[:, t : t + 1]
                )

            def x_sqb(xc):
                # DVE batched row norms from the bf16 (-2x) tiles
                sl = slice(xc * XCH_T, (xc + 1) * XCH_T)
                sq = sqp.tile([P, XCH_T, D], BF16, tag="sqv")
                nc.vector.tensor_tensor(sq[:], xbf[:, sl, :], xbf[:, sl, :], ALU.mult)
                x4 = sqp.tile([P, XCH_T], FP32, tag="x4")
                nc.vector.tensor_reduce(
                    x4[:], sq[:], axis=mybir.AxisListType.X, op=ALU.add
                )
                nc.vector.tensor_scalar_mul(xsq[:, sl], x4[:], 0.25)

            # ---- y chunk staging ----
            ystg = {}

            def y_load(ch, eng=None):
                cols = slice(ch * YCH, (ch + 1) * YCH)
                yst = ystage.tile([P, 2, YCH], FP32, tag="yst")
                (eng or nc.sync).dma_start(
                    yst[:], yt_d[:, cols].rearrange("(h p) c -> p h c", p=P)
                )
                ystg[ch] = yst

            def y_quant(ch, half=None):
                if half is None:
                    cols = slice(ch * YCH, (ch + 1) * YCH)
                    nc.scalar.copy(yT8[:, :, cols], ystg[ch][:])
                else:
                    c0 = half * (YCH // 2)
                    cols = slice(ch * YCH + c0, ch * YCH + c0 + YCH // 2)
                    nc.scalar.copy(
                        yT8[:, :, cols], ystg[ch][:, :, c0 : c0 + YCH // 2]
                    )

            _tsq_tmp = {}

            def y_tsq(ch, step):
                if step < 2:
                    yst = ystg[ch]
                    t = sqp.tile([P, YCH], BF16, tag=f"t{step}")
                    nc.vector.tensor_tensor(
                        t[:], yst[:, step, :], yst[:, step, :], ALU.mult
                    )
                    _tsq_tmp[(ch, step)] = t
                else:
                    cols = slice(ch * YCH, (ch + 1) * YCH)
                    nc.vector.tensor_tensor(
                        yTsq[:, cols],
                        _tsq_tmp.pop((ch, 0))[:],
                        _tsq_tmp.pop((ch, 1))[:],
                        ALU.add,
                    )

            # ---- head ----
            # ALL loads on the sync ring, serialized in priority order -
            # concurrent rings round-robin at packet level and starve
            # the critical path (splitting even the two first critical
            # loads across rings measured ~1.5us WORSE).
            x_load(0)
            y_load(0)
            y_load(1)
            x_load(1)
            x_load(2)
            x_load(3)
            y_load(2)
            y_load(3)
            # PE HAM warm-up: ~3.4us of dummy matmuls in the idle window
            # before data arrives, so transposes + first groups run at
            # 2.4GHz instead of the cold 1.2GHz default. Output discarded.
            warm = psmm.tile([P, P], FP32, tag="mm", name="warm")
            for _ in range(32):
                nc.tensor.matmul(warm[:], ones[:], identity[:], start=True, stop=True)
            # ScalarE chain in dependency-arrival order; quants of the
            # first chunks split 512-col so the first DR isn't gated on
            # a 1.9us wide op.
            x_cast(0)
            y_quant(0, 0)
            y_quant(0, 1)
            x_tr(0)
            y_quant(1, 0)
            y_quant(1, 1)
            x_sq(0)
            x_sq(1)
            # DVE head chain:
            y_tsq(0, 0)
            y_tsq(0, 1)
            y_tsq(0, 2)
            y_tsq(1, 0)
            y_tsq(1, 1)
            y_tsq(1, 2)

            # q0 per-group prep inserts. Deadlines: xT8 m4-7 by g4,
            # m8-11 by g8, m12-15 by g12; xsq[m] by g(m); yT8/yTsq
            # ch2-3 by g16. sqb/tsq on DVE, the rest ScalarE.
            # NOTE: a reader emitted BEFORE its writer in program order
            # reads garbage silently (Tile only orders against already-
            # emitted writers) - every sqb/tr must come after its cast.
            inserts = {
                1: [("sq", 2), ("sq", 3)],
                2: [("cast", 1)],
                3: [("tr", 1), ("sqb", 1)],
                4: [("cast", 2)],
                5: [("tr", 2), ("tsq", 2, 0)],
                6: [("sq", 8), ("tsq", 2, 1)],
                7: [("sq", 9), ("tsq", 2, 2)],
                8: [("sq", 10), ("cast", 3)],
                9: [("sq", 11), ("tr", 3)],
                10: [("sqb", 3)],
                11: [("quant", 2), ("tsq", 3, 0)],
                12: [("tsq", 3, 1)],
                13: [("quant", 3), ("tsq", 3, 2)],
            }

            def run_insert(ins):
                kind = ins[0]
                if kind == "cast":
                    x_cast(ins[1])
                elif kind == "sq":
                    x_sq(ins[1])
                elif kind == "sqb":
                    x_sqb(ins[1])
                elif kind == "tr":
                    x_tr(ins[1])
                elif kind == "quant":
                    y_quant(ins[1])
                elif kind == "tsq":
                    y_tsq(ins[1], ins[2])

            # ---- main loop ----
            for q in range(Q):
                for m in range(M_TILES):
                    if q == 0:
                        for ins in inserts.get(m, []):
                            run_insert(ins)
                    lhs8 = xT8[:, :, m * P : (m + 1) * P]
                    pmA = psmm.tile([P, 2 * NT], FP32, tag="mm", name=f"pa_{q}_{m}")
                    pmB = psmm.tile([P, 2 * NT], FP32, tag="mm", name=f"pb_{q}_{m}")
                    for k in range(GRP):
                        n = q * GRP + k
                        pm = pmA if k < 2 else pmB
                        nc.tensor.matmul(
                            pm[:, (k % 2) * NT : (k % 2 + 1) * NT],
                            lhs8,
                            yT8[:, :, n * NT : (n + 1) * NT],
                            perf_mode=DR,
                            start=True,
                            stop=False,
                        )
                    for k in range(GRP):
                        n = q * GRP + k
                        pm = pmA if k < 2 else pmB
                        nc.tensor.matmul(
                            pm[:, (k % 2) * NT : (k % 2 + 1) * NT],
                            ones[:],
                            yTsq[:, n * NT : (n + 1) * NT],
                            start=False,
                            stop=True,
                        )
                    ot = outp.tile([P, QCOLS], FP16, tag="ot")
                    nc.scalar.activation(
                        ot[:, : 2 * NT],
                        pmA[:],
                        AF.Identity,
                        bias=xsq[:, m : m + 1],
                        scale=1.0,
                    )
                    nc.vector.tensor_scalar_add(
                        ot[:, 2 * NT :], pmB[:], xsq[:, m : m + 1]
                    )
                    nc.sync.dma_start(
                        out_r[:, m, q * QCOLS : (q + 1) * QCOLS], ot[:]
                    )

    nc.compile()
    return nc


def _get_nc():
    if "nc" not in _CACHE:
        _CACHE["nc"] = _build()
    return _CACHE["nc"]


def kernel(x: np.ndarray, y: np.ndarray) -> np.ndarray:
    global LAST_RESULTS
    x = np.ascontiguousarray(np.asarray(x, dtype=np.float32))
    y = np.ascontiguousarray(np.asarray(y, dtype=np.float32))
    assert x.shape == (N_FULL, D) and y.shape == (M_FULL, D)

    nc = _get_nc()
    yt = y.T  # [D, M_FULL], layout prep only
    yhalves = [
        np.ascontiguousarray(yt[:, c * M_SHARD : (c + 1) * M_SHARD])
        for c in range(C_SHARDS)
    ]
    in_maps = []
    for core in range(N_CORES):
        r, c = divmod(core, C_SHARDS)
        in_maps.append({"x": x[r * N_SHARD : (r + 1) * N_SHARD], "yt": yhalves[c]})
    res = run_bass_kernel_spmd(
        nc,
        in_maps,
        core_ids=list(range(N_CORES)),
        trace=bool(os.environ.get("BASS_KERNEL_TRACE")),
    )
    LAST_RESULTS = res
    out = np.empty((N_FULL, M_FULL), dtype=np.float32)
    for core in range(N_CORES):
        r, c = divmod(core, C_SHARDS)
        out[r * N_SHARD : (r + 1) * N_SHARD, c * M_SHARD : (c + 1) * M_SHARD] = (
            res.results[core]["out"].astype(np.float32)
        )
    return out
